# revision 9
# baseline (speedup 1.0000x reference)
"""Farthest-point-sampling (npoint=2) Bass kernel for Trainium2.

Problem: xyz [1, 64, 3, 262144] fp32 -> indices [64, 2] (int64 on host).
Per batch b:
  idx0 = argmax_n y[n]            (y = coord plane 1)
  c    = (x,y,z)[idx0]
  idx1 = argmax_n ((x-cx)^2 + (y-cy)^2 + (z-cz)^2)
argmax = first occurrence on ties (jnp.argmax semantics).

Sharding: data-parallel over batch; 8 NeuronCores x 8 batches each.

Per-core structure, software-pipelined over batches (planes [128,2048]):
  DMA lookahead: y two batches ahead, z one ahead, x just-in-time, so
  the HWDGE queue drains 24 MB back-to-back (~66 us at ~360 GB/s).
  Stage A(b): VectorE Max8+MaxIndex on y; per-batch finale (PE
    transpose of per-partition (max, N-idx) columns -> V reduce +
    is_eq*cand reduce -> idx0); centroid indirect-DMA gather; negate;
    broadcast; ScalarE Square(y + (-cy)).
  Stage B(b): ScalarE Square z/x; GpSimd s1 = sqy+sqz; VectorE
    tensor_tensor_reduce fuses s2 = s1+sqx WITH the row-max accum
    (one pass); MaxIndex with broadcast row-max; dist finale -> idx1.
  Engine balance per batch ~= DMA budget (8.3 us): V 4 full scans +
  4 tiny reduces; G add + casts/stt smalls + gather; S 3 squares +
  copies; idx = N - cand fused into the ScalarE output copy
  (Copy scale=-1 bias=N, f32->i32).
"""

import numpy as np

import concourse.bacc as bacc
import concourse.bass as bass
import concourse.mybir as mybir
from concourse.masks import make_identity
from concourse.tile import TileContext

B = 64  # full batch
N_CORES = 8
BPC = B // N_CORES  # batches per core
N = 262144
P = 128
COLS = N // P  # 2048
BIGK = float(N)

F32 = mybir.dt.float32
U32 = mybir.dt.uint32
I32 = mybir.dt.int32
AX = mybir.AxisListType.X
OP = mybir.AluOpType
SQUARE = mybir.ActivationFunctionType.Square
COPY = mybir.ActivationFunctionType.Copy


import os
VARIANT_TTR = os.environ.get("V_TTR", "split")
CV = int(os.environ.get("V_CV", "640"))  # cols of the s2 add done on V
VARIANT_FUSEOUT = os.environ.get("V_FUSEOUT", "1") == "1"
VARIANT_FUSECAND = os.environ.get("V_FUSECAND", "1") == "1"


def build_nc():
    nc = bacc.Bacc()
    xin = nc.dram_tensor("xyz", [BPC, 3, N], F32, kind="ExternalInput")
    out = nc.dram_tensor("idx", [1, 2 * BPC], I32, kind="ExternalOutput")
    xflat = xin.rearrange("b c n -> (b c n)")[:, None]

    with TileContext(nc) as tc:
        with (
            tc.tile_pool(name="consts", bufs=1) as consts,
            tc.tile_pool(name="yp", bufs=4) as yp,
            tc.tile_pool(name="zp", bufs=3) as zp,
            tc.tile_pool(name="xp", bufs=3) as xp,
            tc.tile_pool(name="sq", bufs=2) as sq,
            tc.tile_pool(name="m8", bufs=4) as m8,
            tc.tile_pool(name="sm", bufs=4) as sm,
            tc.tile_pool(name="acc", bufs=1) as acc,
            tc.tile_pool(name="psT", bufs=2, space="PSUM") as psT,
            tc.tile_pool(name="psS", bufs=1, space="PSUM") as psS,
        ):
            # ---- constants ----
            ident = consts.tile([P, P], F32)
            make_identity(nc, ident)
            ones = consts.tile([1, P], F32)
            nc.vector.memset(ones, 1.0)
            # revb[p] = N - p*COLS ; pbase[c] = c*N   (exact in f32 < 2^24)
            revb_i = consts.tile([P, 1], I32)
            nc.gpsimd.iota(revb_i, pattern=[[0, 1]], base=N, channel_multiplier=-COLS)
            revb_f = consts.tile([P, 1], F32)
            nc.vector.tensor_copy(revb_f, revb_i)
            pbase = consts.tile([3, 1], I32)
            nc.gpsimd.iota(pbase, pattern=[[0, 1]], base=0, channel_multiplier=N)

            out_i = acc.tile([1, 2 * BPC], I32)  # cols 0..7 idx0, 8..15 idx1

            tys = [None] * BPC
            tzs = [None] * BPC
            txs = [None] * BPC
            sys_ = [None] * BPC
            negcs = [None] * BPC

            def dma_y(b):
                t = yp.tile([P, COLS], F32, tag="ty")
                tys[b] = t
                nc.sync.dma_start(t, xin[b, 1].rearrange("(p m) -> p m", p=P))

            def dma_z(b):
                t = zp.tile([P, COLS], F32, tag="tz")
                tzs[b] = t
                nc.sync.dma_start(t, xin[b, 2].rearrange("(p m) -> p m", p=P))

            def dma_x(b):
                t = xp.tile([P, COLS], F32, tag="tx")
                txs[b] = t
                nc.sync.dma_start(t, xin[b, 0].rearrange("(p m) -> p m", p=P))

            def finale(vals_col, idx8_u32, out_col, tagp):
                """vals_col [P,1] f32 per-partition max; idx8_u32 [P,8] u32
                per-partition first-index. Writes N-1-... no: writes
                argmax (first occurrence) as i32 into out_i[:, out_col],
                and returns the [1,1] f32 winning-cand tile (= N - idx)."""
                # cand = revb - col  (u32 col cast to f32 by tensor_scalar)
                cand = sm.tile([P, 1], F32, tag=f"cand{tagp}")
                if VARIANT_FUSECAND:
                    nc.vector.tensor_scalar(
                        out=cand, in0=idx8_u32[:, 0:1], scalar1=-1.0,
                        scalar2=revb_f, op0=OP.mult, op1=OP.add,
                    )
                else:
                    candc = sm.tile([P, 1], F32, tag=f"candc{tagp}")
                    nc.vector.tensor_copy(candc, idx8_u32[:, 0:1])
                    nc.vector.tensor_sub(cand, revb_f, candc)
                pt = psT.tile([1, 2 * P], F32, tag=f"pt{tagp}")
                nc.tensor.transpose(pt[0:1, 0:P], vals_col, ident)
                nc.tensor.transpose(pt[0:1, P : 2 * P], cand, ident)
                rows = sm.tile([1, 2 * P], F32, tag=f"rows{tagp}")
                nc.scalar.copy(rows, pt)
                mx = sm.tile([1, 1], F32, tag=f"mx{tagp}")
                nc.vector.tensor_reduce(mx, rows[:, 0:P], axis=AX, op=OP.max)
                cands = sm.tile([1, P], F32, tag=f"cands{tagp}")
                nc.vector.scalar_tensor_tensor(
                    out=cands, in0=rows[:, 0:P], scalar=mx[0:1, 0:1],
                    in1=rows[:, P : 2 * P], op0=OP.is_equal, op1=OP.mult,
                )
                wc = sm.tile([1, 1], F32, tag=f"wc{tagp}")
                nc.vector.tensor_reduce(wc, cands, axis=AX, op=OP.max)
                if VARIANT_FUSEOUT:
                    # idx = N - wc, fused into the f32->i32 output copy
                    nc.scalar.activation(
                        out_i[0:1, out_col : out_col + 1], wc, COPY,
                        bias=BIGK, scale=-1.0,
                    )
                else:
                    idxf = sm.tile([1, 1], F32, tag=f"idxf{tagp}")
                    nc.vector.tensor_scalar(
                        out=idxf, in0=wc, scalar1=-1.0, scalar2=BIGK,
                        op0=OP.mult, op1=OP.add,
                    )
                    nc.scalar.copy(out_i[0:1, out_col : out_col + 1], idxf)
                return wc

            def stageA(b):
                ty = tys[b]
                ym8 = m8.tile([P, 8], F32, tag="ym8")
                nc.vector.max(out=ym8, in_=ty)
                yi8 = m8.tile([P, 8], U32, tag="yi8")
                nc.vector.max_index(yi8, ym8, ty)
                wc = finale(ym8[:, 0:1], yi8, b, "y")
                # --- centroid gather: idx0 = N - wc (recomputed on V) ---
                idx0f = sm.tile([1, 1], F32, tag="idx0f")
                nc.vector.tensor_scalar(
                    out=idx0f, in0=wc, scalar1=-1.0, scalar2=BIGK,
                    op0=OP.mult, op1=OP.add,
                )
                p3 = psS.tile([3, 1], F32, tag="p3")
                nc.tensor.matmul(
                    p3, ones[0:1, 0:3], idx0f, start=True, stop=True
                )
                offs = sm.tile([3, 1], U32, tag="offs")
                # offs[c] = idx0 + b*3N + c*N (flat index into xin)
                # (on V: GpSimd cannot read PSUM, and p3 lives there)
                nc.vector.scalar_tensor_tensor(
                    out=offs, in0=p3, scalar=float(b * 3 * N), in1=pbase,
                    op0=OP.add, op1=OP.add,
                )
                c3 = sm.tile([3, 1], F32, tag="c3")
                nc.gpsimd.indirect_dma_start(
                    out=c3,
                    out_offset=None,
                    in_=xflat,
                    in_offset=bass.IndirectOffsetOnAxis(ap=offs[0:3, 0:1], axis=0),
                )
                pc3 = psS.tile([1, 3], F32, tag="pc3")
                nc.tensor.transpose(pc3, c3, ident[0:3, 0:3])
                negrow = sm.tile([1, 3], F32, tag="negrow")
                nc.scalar.mul(negrow, pc3, -1.0)
                pnegc = psS.tile([P, 3], F32, tag="pnegc")
                nc.tensor.matmul(pnegc, ones, negrow, start=True, stop=True)
                negc = sm.tile([P, 3], F32, tag="negc")
                nc.scalar.copy(negc, pnegc)
                negcs[b] = negc
                # sy early: y tile + centroid both ready before z lands
                sy = sq.tile([P, COLS], F32, tag="sy")
                nc.scalar.activation(sy, ty, SQUARE, bias=negc[:, 1:2])
                sys_[b] = sy
                tys[b] = None

            def stageB(b):
                negc = negcs[b]
                sz = sq.tile([P, COLS], F32, tag="sz")
                nc.scalar.activation(sz, tzs[b], SQUARE, bias=negc[:, 2:3])
                s1 = sq.tile([P, COLS], F32, tag="s1")
                nc.gpsimd.tensor_add(s1, sys_[b], sz)
                sx = sq.tile([P, COLS], F32, tag="sx")
                nc.scalar.activation(sx, txs[b], SQUARE, bias=negc[:, 0:1])
                s2 = sq.tile([P, COLS], F32, tag="s2")
                dmx8 = m8.tile([P, 8], F32, tag="dmx8")
                if VARIANT_TTR == "split":
                    # s2 = s1 + sx, V takes the first CV cols, G the rest
                    nc.vector.tensor_add(s2[:, 0:CV], s1[:, 0:CV], sx[:, 0:CV])
                    nc.gpsimd.tensor_add(s2[:, CV:], s1[:, CV:], sx[:, CV:])
                else:
                    nc.gpsimd.tensor_add(s2, s1, sx)
                nc.vector.max(out=dmx8, in_=s2)
                di8 = m8.tile([P, 8], U32, tag="di8")
                nc.vector.max_index(di8, dmx8, s2)
                finale(dmx8[:, 0:1], di8, BPC + b, "d")
                tzs[b] = None
                txs[b] = None
                sys_[b] = None

            # ---- software pipeline ----
            dma_y(0)
            dma_y(1)
            dma_z(0)
            for i in range(BPC + 1):
                if i < BPC:
                    dma_x(i)
                    if i + 2 < BPC:
                        dma_y(i + 2)
                    if i + 1 < BPC:
                        dma_z(i + 1)
                    stageA(i)
                if i >= 1:
                    stageB(i - 1)

            nc.sync.dma_start(out[:, :], out_i[:, :])

    nc.compile()
    return nc


_NC_CACHE = None


def _get_nc():
    global _NC_CACHE
    if _NC_CACHE is None:
        _NC_CACHE = build_nc()
    return _NC_CACHE


def kernel(xyz: np.ndarray) -> np.ndarray:
    from concourse.bass_utils import run_bass_kernel_spmd

    assert xyz.shape == (1, B, 3, N), xyz.shape
    xyz = np.ascontiguousarray(xyz, dtype=np.float32)
    nc = _get_nc()
    in_maps = [
        {"xyz": np.ascontiguousarray(xyz[0, k * BPC : (k + 1) * BPC])}
        for k in range(N_CORES)
    ]
    res = run_bass_kernel_spmd(nc, in_maps, core_ids=list(range(N_CORES)))
    # out layout per core: [1, 16] = [idx0 x8 | idx1 x8]
    outs = [res.results[k]["idx"].reshape(2, BPC).T for k in range(N_CORES)]
    return np.concatenate(outs, axis=0).astype(np.int64)
